# revision 29
# baseline (speedup 1.0000x reference)
"""Trainium2 Bass kernel for nn_ModelIAS_53618371724066 (segment_reduce).

Computes, for each batch row b:
    logits = hidden[b, 1:, :] @ W + b_vec          # [T, S]
    merged[w, :] = mean over {t : seg[b,t] == w} of logits[t, :]   (0 if empty)
    out[b] = merged.T                               # [S, T]

Strategy (data-parallel over batch, 32 rows per core on 8 cores):
  - Host prep is layout/precision only: hidden is transposed to a
    partition-major [p, row, k, t] layout and quantized to fp8 e3m4 (4
    mantissa bits; |h| < 15.5 so range is safe) -- this HALVES the input
    HBM traffic vs fp16 and measures rel_err 1.41e-2 on hardware, inside
    the 2e-2 gate with margin; the PE runs mixed fp8e3-stationary x
    fp16-moving matmuls with fp32 PSUM accumulation.  W stays fp16 (fp8 W
    would push the error past the gate).
  - The mean-weighted segment matrix Mg[t, w] = g[t] * (seg[t] == w) with
    g[t] = 1/count[seg[t]] is built on-chip in fp16 with one dual-op DVE
    tensor_scalar per t-chunk (is_equal then mult, both per-partition
    scalars), so the mean normalization costs nothing extra.
  - Stage 1 (PE): logits[t_chunk, s] = sum_k hiddenT[k-chunk].T @ W[k-chunk]
    accumulated in fp32 PSUM; both t-chunks land in ONE PSUM tile
    [128, 2, S] so the PSUM->SBUF fp16 evacuation is a single ACT copy.
    Bias is folded in as a rank-1 matmul when b != 0.
  - Stage 2 (PE): out[w, s] = sum_c Mg[:, c, wchunk].T @ lsb[:, c, :] with
    Mg STATIONARY and the fp16 logits moving (N=130 stream, not 256) —
    this is ~2x fewer PE streaming cycles than the lsb-stationary
    formulation and has no wasted [128,2]-stationary tail matmul.
  - Both w-chunks of stage 2 accumulate into one PSUM tile [128, 2, S];
    ACT casts it to fp16 in a single copy (DMA cannot read PSUM, and the
    Pool/gpsimd engine cannot touch PSUM either), and output DMAs go out
    2 rows at a time (fp16, half the bytes of fp32) with 1 KB contiguous
    runs per partition.  Host reassembles [w, s] -> [s, t].
  - Stage 2 of row r-1 is emitted on the PE queue AFTER stage 1 of row r
    (one-row software pipeline) so the in-order PE never stalls on the
    ACT-produced lsb of the same row.  Per-row engine budget: PE ~0.94us,
    ACT ~0.95us (lsb + output cast), DVE ~0.82us (2x Mg build).
  - Hidden streams on the GpSimd ring in 1-row DMAs (SWDGE; moving it to
    the SP HWDGE ring oversubscribes that sequencer and measures slower)
    while outputs and constants use the SP ring; per-instruction sem-waits are legalized for
    the pinned walrus by _split_sync_waits.
"""

import numpy as np

import concourse.bass as bass
import concourse.tile as tile
from concourse import mybir
from concourse.bass_utils import run_bass_kernel_spmd

B, T, H, S = 256, 256, 768, 130
N_CORES = 8
RPC = B // N_CORES  # rows per core
KCH = H // 128  # k chunks of the hidden dim
F32 = mybir.dt.float32
HP = mybir.dt.float16
H8 = mybir.dt.float8e3  # e3m4: 4 mantissa bits, covers |h|<~15.5


def _split_sync_waits(nc):
    """The pinned walrus build rejects instructions carrying more than one
    sync-wait command ("Too many sync wait commands", setupSyncWait).  Keep
    one wait per instruction and hoist the rest onto NoOps inserted just
    before it on the same engine (same semantics: all waits still execute
    before the instruction, in stream order)."""
    for f in nc.m.functions:
        for blk in f.blocks:
            il = blk.instructions
            i = 0
            while i < len(il):
                inst = il[i]
                si = inst.sync_info
                if si is not None and si.on_wait and len(si.on_wait) >= 2:
                    waits = list(si.on_wait)
                    keep = [waits.pop()]
                    pos = i
                    for j, w in enumerate(waits):
                        nop = mybir.InstNoOp(name=f"{inst.name}_ws{j}", ins=[], outs=[])
                        nop.engine = inst.engine
                        nop.sync_info = mybir.SyncInfo(on_wait=[w], on_update=[])
                        il.insert(pos, nop)
                        pos += 1
                        i += 1
                    inst.sync_info = mybir.SyncInfo(
                        on_wait=keep, on_update=list(si.on_update)
                    )
                i += 1


def _build_program(rpc=RPC, with_bias=False, hid_bufs=10, split_waits=True):
    nc = bass.Bass("TRN2", target_bir_lowering=False, debug=False)

    hid = nc.dram_tensor("hiddent", [128, rpc, KCH, T], H8, kind="ExternalInput")
    w_d = nc.dram_tensor("w", [128, KCH, S], HP, kind="ExternalInput")
    b_d = nc.dram_tensor("bvec", [1, S], HP, kind="ExternalInput")
    seg_d = nc.dram_tensor("segt", [128, 2, rpc], F32, kind="ExternalInput")
    g_d = nc.dram_tensor("gt", [128, 2, rpc], F32, kind="ExternalInput")
    # [w_partition, row, w_chunk, s] fp16; host reassembles to [B, S, T]
    out_d = nc.dram_tensor("out", [128, rpc, 2, S], HP, kind="ExternalOutput")

    eq = mybir.AluOpType.is_equal
    mult = mybir.AluOpType.mult
    assert rpc % 2 == 0
    with tile.TileContext(nc) as tc:
        with (
            tc.tile_pool(name="const", bufs=1) as const_pool,
            tc.tile_pool(name="hid", bufs=hid_bufs) as hid_pool,
            tc.tile_pool(name="mbar", bufs=3) as m_pool,
            tc.tile_pool(name="lsb", bufs=3) as l_pool,
            tc.tile_pool(name="osb", bufs=4) as o_pool,
            tc.tile_pool(name="psl", bufs=3, space=bass.MemorySpace.PSUM) as psl_pool,
            tc.tile_pool(name="pso", bufs=4, space=bass.MemorySpace.PSUM) as pso_pool,
            tc.tile_pool(name="wu", bufs=1, space=bass.MemorySpace.PSUM) as wu_pool,
        ):
            # --- constants; hidden rows stream in 1-row fp8 DMAs on the
            # gpsimd ring (~0.2MB each), prefetched 2 rows ahead ---
            hts = {}
            obs = {}

            def fetch_row(rr_, chunks=((0, KCH),)):
                t = hid_pool.tile([128, KCH, T], H8, tag="ht", name="ht")
                for j0, j1 in chunks:
                    nc.gpsimd.dma_start(t[:, j0:j1], hid.ap()[:, rr_, j0:j1])
                hts[rr_] = t

            # row 0 lands k-chunk 0 first so the PE starts ~1.5us earlier
            # (the tile deps are per-DMA, so matmul k=0 only waits chunk 0)
            fetch_row(0, chunks=((0, 1), (1, 3), (3, KCH)))
            wt = const_pool.tile([128, KCH, S], HP)
            nc.sync.dma_start(wt[:], w_d.ap()[:])
            segt = const_pool.tile([128, 2, rpc], F32)
            nc.sync.dma_start(segt[:], seg_d.ap()[:])
            gt = const_pool.tile([128, 2, rpc], F32)
            nc.sync.dma_start(gt[:], g_d.ap()[:])
            iota_i = const_pool.tile([128, T], mybir.dt.int32)
            nc.gpsimd.iota(iota_i[:], pattern=[[1, T]], base=0, channel_multiplier=0)
            iota_f = const_pool.tile([128, T], F32)
            nc.vector.tensor_copy(iota_f[:], iota_i[:])
            if with_bias:
                ones = const_pool.tile([1, 128], HP)
                nc.vector.memset(ones[:], 1.0)
                bsb = const_pool.tile([1, S], HP)
                nc.sync.dma_start(bsb[:], b_d.ap()[:])

            fetch_row(1)

            # ~48 tiny matmuls keep the PE busy from engine-boot so the DVFS
            # ramp (105ns/MM for the first ~3us of activity) completes before
            # the first real data lands
            wu = const_pool.tile([1, 2], HP)
            nc.vector.memset(wu[:], 0.0)
            wup = wu_pool.tile([2, 2], F32, name="wup")
            for _ in range(48):
                nc.tensor.matmul(wup[:], wu[:], wu[:], start=True, stop=True)

            def emit_stage2(item):
                pr, plsb, pmbar = item
                ppair, prr = divmod(pr, 2)
                # out[w, s] = sum_c Mg[:, c, wchunk].T @ lsb[:, c, :] with Mg
                # stationary and the fp16 logits moving (N=130 stream)
                pso = pso_pool.tile([128, 2, S], F32, name="pso")
                for wc in range(2):
                    for c in range(2):
                        nc.tensor.matmul(
                            pso[:, wc, :],
                            pmbar[:, c, 128 * wc : 128 * (wc + 1)],
                            plsb[:, c, :],
                            start=(c == 0),
                            stop=(c == 1),
                        )
                # PSUM -> SBUF fp16 on ACT; DMA out every 2 rows on SP
                if prr == 0:
                    obs[ppair] = o_pool.tile([128, 2, 2, S], HP, tag="ob", name="ob")
                ob = obs[ppair]
                nc.scalar.copy(ob[:, prr], pso[:])
                if prr == 1:
                    nc.sync.dma_start(out_d.ap()[:, 2 * ppair : 2 * ppair + 2], ob[:])

            pending = None
            for r in range(rpc):
                if r + 2 < rpc:
                    fetch_row(r + 2)
                ht = hts.pop(r)

                # Mg[t, w] = (seg[t] == w) * g[t], fp16, t-chunked, on DVE
                # (gpsimd tensor_scalar is a ~4us DSP program -- never use it)
                mbar = m_pool.tile([128, 2, T], HP)
                for c in range(2):
                    nc.vector.tensor_scalar(
                        mbar[:, c, :],
                        iota_f[:],
                        segt[:, c, r : r + 1],
                        gt[:, c, r : r + 1],
                        eq,
                        mult,
                    )

                # stage 1: logits for both t-chunks into one fp32 PSUM tile
                psl = psl_pool.tile([128, 2, S], F32)
                for c in range(2):
                    for k in range(KCH):
                        nc.tensor.matmul(
                            psl[:, c, :],
                            ht[:, k, 128 * c : 128 * (c + 1)],
                            wt[:, k, :],
                            start=(k == 0),
                            stop=(k == KCH - 1 and not with_bias),
                        )
                    if with_bias:
                        nc.tensor.matmul(
                            psl[:, c, :], ones[:], bsb[:], start=False, stop=True
                        )

                # stage 2 of the PREVIOUS row goes on the PE queue here so the
                # PE never waits on the ACT-produced lsb of the same row
                if pending is not None:
                    emit_stage2(pending)

                # PSUM -> SBUF fp16 in one ACT copy (g lives in Mg, not here)
                lsb = l_pool.tile([128, 2, S], HP)
                nc.scalar.copy(lsb[:], psl[:])
                pending = (r, lsb, mbar)
            emit_stage2(pending)

    if split_waits:
        _split_sync_waits(nc)
    return nc


def _host_prep(hidden, W, b, seg):
    """Pure layout/encoding prep (no float arithmetic on the model data
    beyond 1/count of the integer segment ids)."""
    # [core][p, r, k, t] with p the SBUF partition (= h % 128 within chunk k)
    import ml_dtypes

    h8 = np.asarray(hidden[:, 1:, :], dtype=np.float32).astype(ml_dtypes.float8_e3m4)
    h8 = h8.reshape(N_CORES, RPC, T, KCH, 128)
    hiddenT = np.ascontiguousarray(h8.transpose(0, 4, 1, 3, 2))

    seg = np.asarray(seg)
    counts = np.zeros((B, T), dtype=np.int64)
    rows = np.arange(B)[:, None]
    np.add.at(counts, (rows, seg), 1)
    g = (1.0 / np.maximum(counts, 1))[rows, seg].astype(np.float32)  # [B, T]
    segf = seg.astype(np.float32)

    # partition-major packing: [core][p, c, r] = value at (row0+r, 128c+p)
    def pack(x):
        # x: [B, T] -> [N_CORES, 128, 2, RPC]
        x4 = x.reshape(N_CORES, RPC, 2, 128)  # [core, r, c, p]
        return np.ascontiguousarray(x4.transpose(0, 3, 2, 1))

    segt = pack(segf)
    gt = pack(g)
    w16 = np.asarray(W, dtype=np.float32).astype(np.float16).reshape(KCH, 128, S)
    w_in = np.ascontiguousarray(w16.transpose(1, 0, 2))  # [128, KCH, S]
    b_in = np.ascontiguousarray(b, dtype=np.float32).astype(np.float16).reshape(1, S)
    return hiddenT, w_in, b_in, segt, gt


_CACHE = {}


def kernel(hidden, W, b, seg):
    hiddenT, w_in, b_in, segt, gt = _host_prep(hidden, W, b, seg)
    with_bias = bool(np.any(b_in != 0.0))

    key = ("prog", with_bias)
    if key not in _CACHE:
        _CACHE[key] = _build_program(with_bias=with_bias)
    nc = _CACHE[key]

    in_maps = []
    for c in range(N_CORES):
        in_maps.append(
            {
                "hiddent": hiddenT[c],
                "w": w_in,
                "bvec": b_in,
                "segt": segt[c],
                "gt": gt[c],
            }
        )
    res = run_bass_kernel_spmd(nc, in_maps, core_ids=list(range(N_CORES)))
    # device layout is [w_part=128, RPC, w_chunk=2, S]; out[b, s, 128*wc + p]
    # = dev[p, r, wc, s] -> transpose to [RPC, S, wc, p] and flatten t.
    parts = []
    for c in range(N_CORES):
        dev = res.results[c]["out"]  # [128, RPC, 2, S] fp16
        parts.append(
            dev.transpose(1, 3, 2, 0).reshape(RPC, S, T).astype(np.float32)
        )
    return np.ascontiguousarray(np.concatenate(parts, axis=0))


# revision 30
# speedup vs baseline: 1.0132x; 1.0132x over previous
"""Trainium2 Bass kernel for nn_ModelIAS_53618371724066 (segment_reduce).

Computes, for each batch row b:
    logits = hidden[b, 1:, :] @ W + b_vec          # [T, S]
    merged[w, :] = mean over {t : seg[b,t] == w} of logits[t, :]   (0 if empty)
    out[b] = merged.T                               # [S, T]

Strategy (data-parallel over batch, 32 rows per core on 8 cores):
  - Host prep is layout/precision only: hidden is transposed to a
    partition-major [p, row, k, t] layout and quantized to fp8 e3m4 (4
    mantissa bits; |h| < 15.5 so range is safe) -- this HALVES the input
    HBM traffic vs fp16 and measures rel_err 1.41e-2 on hardware, inside
    the 2e-2 gate with margin; the PE runs mixed fp8e3-stationary x
    fp16-moving matmuls with fp32 PSUM accumulation.  W stays fp16 (fp8 W
    would push the error past the gate).
  - The mean-weighted segment matrix Mg[t, w] = g[t] * (seg[t] == w) with
    g[t] = 1/count[seg[t]] is built on-chip in fp16 with one dual-op DVE
    tensor_scalar per t-chunk (is_equal then mult, both per-partition
    scalars), so the mean normalization costs nothing extra.
  - Stage 1 (PE): logits[t_chunk, s] = sum_k hiddenT[k-chunk].T @ W[k-chunk]
    accumulated in fp32 PSUM; both t-chunks land in ONE PSUM tile
    [128, 2, S] so the PSUM->SBUF fp16 evacuation is a single ACT copy.
    Bias is folded in as a rank-1 matmul when b != 0.
  - Stage 2 (PE): out[w, s] = sum_c Mg[:, c, wchunk].T @ lsb[:, c, :] with
    Mg STATIONARY and the fp16 logits moving (N=130 stream, not 256) —
    this is ~2x fewer PE streaming cycles than the lsb-stationary
    formulation and has no wasted [128,2]-stationary tail matmul.
  - Both w-chunks of stage 2 accumulate into one PSUM tile [128, 2, S];
    ACT casts it to fp16 in a single copy (DMA cannot read PSUM, and the
    Pool/gpsimd engine cannot touch PSUM either), and output DMAs go out
    2 rows at a time (fp16, half the bytes of fp32) with 1 KB contiguous
    runs per partition.  Host reassembles [w, s] -> [s, t].
  - Stage 2 of row r-1 is emitted on the PE queue AFTER stage 1 of row r
    (one-row software pipeline) so the in-order PE never stalls on the
    ACT-produced lsb of the same row.  Per-row engine budget: PE ~0.94us,
    ACT ~0.95us (lsb + output cast), DVE ~0.82us (2x Mg build).
  - Hidden streams on the GpSimd ring in 1-row DMAs (SWDGE; moving it to
    the SP HWDGE ring oversubscribes that sequencer and measures slower)
    while outputs and constants use the SP ring; per-instruction sem-waits are legalized for
    the pinned walrus by _split_sync_waits.
"""

import numpy as np

import concourse.bass as bass
import concourse.tile as tile
from concourse import mybir
from concourse.bass_utils import run_bass_kernel_spmd

B, T, H, S = 256, 256, 768, 130
N_CORES = 8
RPC = B // N_CORES  # rows per core
KCH = H // 128  # k chunks of the hidden dim
F32 = mybir.dt.float32
HP = mybir.dt.float16
H8 = mybir.dt.float8e3  # e3m4: 4 mantissa bits, covers |h|<~15.5


def _split_sync_waits(nc):
    """The pinned walrus build rejects instructions carrying more than one
    sync-wait command ("Too many sync wait commands", setupSyncWait).  Keep
    one wait per instruction and hoist the rest onto NoOps inserted just
    before it on the same engine (same semantics: all waits still execute
    before the instruction, in stream order)."""
    for f in nc.m.functions:
        for blk in f.blocks:
            il = blk.instructions
            i = 0
            while i < len(il):
                inst = il[i]
                si = inst.sync_info
                if si is not None and si.on_wait and len(si.on_wait) >= 2:
                    waits = list(si.on_wait)
                    keep = [waits.pop()]
                    pos = i
                    for j, w in enumerate(waits):
                        nop = mybir.InstNoOp(name=f"{inst.name}_ws{j}", ins=[], outs=[])
                        nop.engine = inst.engine
                        nop.sync_info = mybir.SyncInfo(on_wait=[w], on_update=[])
                        il.insert(pos, nop)
                        pos += 1
                        i += 1
                    inst.sync_info = mybir.SyncInfo(
                        on_wait=keep, on_update=list(si.on_update)
                    )
                i += 1


def _build_program(rpc=RPC, with_bias=False, hid_bufs=10, split_waits=True):
    nc = bass.Bass("TRN2", target_bir_lowering=False, debug=False)

    hid = nc.dram_tensor("hiddent", [128, rpc, KCH, T], H8, kind="ExternalInput")
    w_d = nc.dram_tensor("w", [128, KCH, S], HP, kind="ExternalInput")
    b_d = nc.dram_tensor("bvec", [1, S], HP, kind="ExternalInput")
    seg_d = nc.dram_tensor("segt", [128, 2, rpc], F32, kind="ExternalInput")
    g_d = nc.dram_tensor("gt", [128, 2, rpc], F32, kind="ExternalInput")
    # [w_partition, row, w_chunk, s] fp16; host reassembles to [B, S, T]
    out_d = nc.dram_tensor("out", [128, rpc, 2, S], HP, kind="ExternalOutput")

    eq = mybir.AluOpType.is_equal
    mult = mybir.AluOpType.mult
    assert rpc % 2 == 0
    with tile.TileContext(nc) as tc:
        with (
            tc.tile_pool(name="const", bufs=1) as const_pool,
            tc.tile_pool(name="hid", bufs=hid_bufs) as hid_pool,
            tc.tile_pool(name="mbar", bufs=3) as m_pool,
            tc.tile_pool(name="lsb", bufs=3) as l_pool,
            tc.tile_pool(name="osb", bufs=4) as o_pool,
            tc.tile_pool(name="psl", bufs=3, space=bass.MemorySpace.PSUM) as psl_pool,
            tc.tile_pool(name="pso", bufs=5, space=bass.MemorySpace.PSUM) as pso_pool,
        ):
            # --- constants; hidden rows stream in 1-row fp8 DMAs on the
            # gpsimd ring (~0.2MB each), prefetched 2 rows ahead ---
            hts = {}
            obs = {}

            def fetch_row(rr_, chunks=((0, KCH),)):
                t = hid_pool.tile([128, KCH, T], H8, tag="ht", name="ht")
                for j0, j1 in chunks:
                    nc.gpsimd.dma_start(t[:, j0:j1], hid.ap()[:, rr_, j0:j1])
                hts[rr_] = t

            # row 0 lands k-chunk 0 first so the PE starts ~1.5us earlier
            # (the tile deps are per-DMA, so matmul k=0 only waits chunk 0)
            fetch_row(0, chunks=((0, 1), (1, 3), (3, KCH)))
            wt = const_pool.tile([128, KCH, S], HP)
            nc.sync.dma_start(wt[:], w_d.ap()[:])
            segt = const_pool.tile([128, 2, rpc], F32)
            nc.sync.dma_start(segt[:], seg_d.ap()[:])
            gt = const_pool.tile([128, 2, rpc], F32)
            nc.sync.dma_start(gt[:], g_d.ap()[:])
            iota_i = const_pool.tile([128, T], mybir.dt.int32)
            nc.gpsimd.iota(iota_i[:], pattern=[[1, T]], base=0, channel_multiplier=0)
            iota_f = const_pool.tile([128, T], F32)
            nc.vector.tensor_copy(iota_f[:], iota_i[:])
            if with_bias:
                ones = const_pool.tile([1, 128], HP)
                nc.vector.memset(ones[:], 1.0)
                bsb = const_pool.tile([1, S], HP)
                nc.sync.dma_start(bsb[:], b_d.ap()[:])

            fetch_row(1)

            def emit_stage2(item):
                pr, plsb, pmbar = item
                ppair, prr = divmod(pr, 2)
                # out[w, s] = sum_c Mg[:, c, wchunk].T @ lsb[:, c, :] with Mg
                # stationary and the fp16 logits moving (N=130 stream)
                pso = pso_pool.tile([128, 2, S], F32, name="pso")
                for wc in range(2):
                    for c in range(2):
                        nc.tensor.matmul(
                            pso[:, wc, :],
                            pmbar[:, c, 128 * wc : 128 * (wc + 1)],
                            plsb[:, c, :],
                            start=(c == 0),
                            stop=(c == 1),
                        )
                # PSUM -> SBUF fp16 on ACT; DMA out every 2 rows on SP
                if prr == 0:
                    obs[ppair] = o_pool.tile([128, 2, 2, S], HP, tag="ob", name="ob")
                ob = obs[ppair]
                nc.scalar.copy(ob[:, prr], pso[:])
                if prr == 1:
                    nc.sync.dma_start(out_d.ap()[:, 2 * ppair : 2 * ppair + 2], ob[:])

            pending = None
            for r in range(rpc):
                if r + 2 < rpc:
                    fetch_row(r + 2)
                ht = hts.pop(r)

                # Mg[t, w] = (seg[t] == w) * g[t], fp16, t-chunked, on DVE
                # (gpsimd tensor_scalar is a ~4us DSP program -- never use it)
                mbar = m_pool.tile([128, 2, T], HP)
                for c in range(2):
                    nc.vector.tensor_scalar(
                        mbar[:, c, :],
                        iota_f[:],
                        segt[:, c, r : r + 1],
                        gt[:, c, r : r + 1],
                        eq,
                        mult,
                    )

                # stage 1: logits for both t-chunks into one fp32 PSUM tile
                psl = psl_pool.tile([128, 2, S], F32)
                for c in range(2):
                    for k in range(KCH):
                        nc.tensor.matmul(
                            psl[:, c, :],
                            ht[:, k, 128 * c : 128 * (c + 1)],
                            wt[:, k, :],
                            start=(k == 0),
                            stop=(k == KCH - 1 and not with_bias),
                        )
                    if with_bias:
                        nc.tensor.matmul(
                            psl[:, c, :], ones[:], bsb[:], start=False, stop=True
                        )

                # stage 2 of the PREVIOUS row goes on the PE queue here so the
                # PE never waits on the ACT-produced lsb of the same row
                if pending is not None:
                    emit_stage2(pending)

                # PSUM -> SBUF fp16 in one ACT copy (g lives in Mg, not here)
                lsb = l_pool.tile([128, 2, S], HP)
                nc.scalar.copy(lsb[:], psl[:])
                pending = (r, lsb, mbar)
            emit_stage2(pending)

    if split_waits:
        _split_sync_waits(nc)
    return nc


def _host_prep(hidden, W, b, seg):
    """Pure layout/encoding prep (no float arithmetic on the model data
    beyond 1/count of the integer segment ids)."""
    # [core][p, r, k, t] with p the SBUF partition (= h % 128 within chunk k)
    import ml_dtypes

    h8 = np.asarray(hidden[:, 1:, :], dtype=np.float32).astype(ml_dtypes.float8_e3m4)
    h8 = h8.reshape(N_CORES, RPC, T, KCH, 128)
    hiddenT = np.ascontiguousarray(h8.transpose(0, 4, 1, 3, 2))

    seg = np.asarray(seg)
    counts = np.zeros((B, T), dtype=np.int64)
    rows = np.arange(B)[:, None]
    np.add.at(counts, (rows, seg), 1)
    g = (1.0 / np.maximum(counts, 1))[rows, seg].astype(np.float32)  # [B, T]
    segf = seg.astype(np.float32)

    # partition-major packing: [core][p, c, r] = value at (row0+r, 128c+p)
    def pack(x):
        # x: [B, T] -> [N_CORES, 128, 2, RPC]
        x4 = x.reshape(N_CORES, RPC, 2, 128)  # [core, r, c, p]
        return np.ascontiguousarray(x4.transpose(0, 3, 2, 1))

    segt = pack(segf)
    gt = pack(g)
    w16 = np.asarray(W, dtype=np.float32).astype(np.float16).reshape(KCH, 128, S)
    w_in = np.ascontiguousarray(w16.transpose(1, 0, 2))  # [128, KCH, S]
    b_in = np.ascontiguousarray(b, dtype=np.float32).astype(np.float16).reshape(1, S)
    return hiddenT, w_in, b_in, segt, gt


_CACHE = {}


def kernel(hidden, W, b, seg):
    hiddenT, w_in, b_in, segt, gt = _host_prep(hidden, W, b, seg)
    with_bias = bool(np.any(b_in != 0.0))

    key = ("prog", with_bias)
    if key not in _CACHE:
        _CACHE[key] = _build_program(with_bias=with_bias)
    nc = _CACHE[key]

    in_maps = []
    for c in range(N_CORES):
        in_maps.append(
            {
                "hiddent": hiddenT[c],
                "w": w_in,
                "bvec": b_in,
                "segt": segt[c],
                "gt": gt[c],
            }
        )
    res = run_bass_kernel_spmd(nc, in_maps, core_ids=list(range(N_CORES)))
    # device layout is [w_part=128, RPC, w_chunk=2, S]; out[b, s, 128*wc + p]
    # = dev[p, r, wc, s] -> transpose to [RPC, S, wc, p] and flatten t.
    parts = []
    for c in range(N_CORES):
        dev = res.results[c]["out"]  # [128, RPC, 2, S] fp16
        parts.append(
            dev.transpose(1, 3, 2, 0).reshape(RPC, S, T).astype(np.float32)
        )
    return np.ascontiguousarray(np.concatenate(parts, axis=0))
